# revision 49
# baseline (speedup 1.0000x reference)
"""MultiHeadAttention Trainium2 Bass kernel.

Head-sharded tensor parallel across 8 NeuronCores (2 heads/core).
All-transposed dataflow: activations live feature-on-partition so no
on-device activation transposes are needed; the per-head attention
computes S.T = K Q.T directly, softmax is max-free (scores are bounded),
the additive attention bias is applied as a multiply by exp(bias)
(precomputed on host), and the key-padding mask is applied by HOST-SIDE
COMPACTION: masked key/value positions are gathered out before the
kernel runs (they contribute exactly zero to the softmax numerator and
denominator), the survivors are zero-padded up to a multiple of 128,
and padding columns are neutralized by exp(bias)=0.  With a ~50% random
mask this halves the k-side work (scores, exp, bias-multiply, PV) and
the k-side DMA.  The module is compiled per compacted-k-block-count
(KBE) at call time.

Host side: inputs are pre-transposed / pre-cast to fp16, outputs are
partial sums (row-parallel out projection) summed on host.
"""

import sys

sys.path.insert(0, "/opt/trn_rl_repo")

import numpy as np

B, S, H, NH = 2, 2048, 1024, 16
HD = H // NH            # 64
NCORES = 8
HPC = NH // NCORES      # 2 heads per core
CW = HPC * HD           # 128 = per-core slice width
R = B * S               # 4096 flattened q rows
SCALE = float(HD) ** -0.5
F = H // 128            # 8 feature blocks
QC = S // 512           # 4 q chunks per batch
DEFAULT_KBE = 9         # compacted k blocks per batch (runtime-derived)

_CACHE = {}
_LAST_KBE = None


def _kbe_for(key_padding_mask):
    kpm = np.asarray(key_padding_mask)
    cnt = (~kpm).sum(axis=1).max()
    return max(int(-(-int(cnt) // 128)), 1)


def _chunks(sp):
    """Split sp columns into DMA/proj chunks of <=512."""
    out, o = [], 0
    while o < sp:
        w = min(512, sp - o)
        out.append((o, w))
        o += w
    return out


def _granules(kbe):
    """Split kbe k-blocks into eb-load granules of <=3."""
    out, g = [], 0
    while g < kbe:
        n = min(3, kbe - g)
        out.append((g, n))
        g += n
    return out


def _build_module(reps=1, kbe=DEFAULT_KBE):
    import concourse.bass as bass
    import concourse.tile as tile
    from concourse import bacc, mybir
    from concourse.masks import make_identity

    f16 = mybir.dt.float16
    f32 = mybir.dt.float32
    Exp = mybir.ActivationFunctionType.Exp

    sp = kbe * 128
    rk = B * sp

    nc = bacc.Bacc(
        "TRN2", target_bir_lowering=False, debug=False, num_devices=NCORES
    )

    # ---- DRAM I/O (per core) ----
    xq = nc.dram_tensor("xq_t", [H, R], f16, kind="ExternalInput").ap()
    xk = nc.dram_tensor("xk_t", [H, rk], f16, kind="ExternalInput").ap()
    xv = nc.dram_tensor("xv_t", [H, rk], f16, kind="ExternalInput").ap()
    wq = nc.dram_tensor("wq_t", [H, CW], f16, kind="ExternalInput").ap()
    wk = nc.dram_tensor("wk_t", [H, CW], f16, kind="ExternalInput").ap()
    wv = nc.dram_tensor("wv_t", [H, CW], f16, kind="ExternalInput").ap()
    wo = nc.dram_tensor("wo_t", [CW, H], f16, kind="ExternalInput").ap()
    qb = nc.dram_tensor("qb_col", [CW, 1], f32, kind="ExternalInput").ap()
    kb_ = nc.dram_tensor("kb_col", [CW, 1], f32, kind="ExternalInput").ap()
    eb = nc.dram_tensor("eb_t", [B, QC, sp, HPC * 512], f16,
                        kind="ExternalInput").ap()
    m01f = nc.dram_tensor("m01_f32", [128, B * kbe], f32,
                          kind="ExternalInput").ap()
    m01h = nc.dram_tensor("m01_v", [128, B * kbe], f16,
                          kind="ExternalInput").ap()
    opart = nc.dram_tensor("o_part", [R, H], f16, kind="ExternalOutput").ap()

    with tile.TileContext(nc) as tc:
        for _ in range(reps):
            _emit(tc, nc, f16, f32, Exp, make_identity, kbe,
                  xq, xk, xv, wq, wk, wv, wo, qb, kb_, eb, m01f, m01h, opart)

    nc.compile()
    return nc


def _emit(tc, nc, f16, f32, Exp, make_identity, kbe,
          xq, xk, xv, wq, wk, wv, wo, qb, kb_, eb, m01f, m01h, opart):
    from contextlib import ExitStack

    sp = kbe * 128
    tt = B * kbe
    kchunks = _chunks(sp)
    grans = _granules(kbe)
    # The tuned schedule's deep PV pipe + big eb tiles only fit SBUF for
    # the compacted case; with little/no masking fall back to a simple
    # sequential schedule with small pools.
    deep = kbe <= 11

    with ExitStack() as top:
        consts = top.enter_context(tc.tile_pool(name="consts", bufs=1))
        pers = top.enter_context(tc.tile_pool(name="pers", bufs=1))
        xpool = top.enter_context(tc.tile_pool(name="xin", bufs=4))
        mm = top.enter_context(tc.tile_pool(name="mmpsum", bufs=3,
                                            space="PSUM"))
        cvp_pool = top.enter_context(tc.tile_pool(name="cvpsum", bufs=2,
                                                  space="PSUM"))
        vtp = top.enter_context(tc.tile_pool(name="vt", bufs=2))
        ebp = top.enter_context(tc.tile_pool(name="ebp", bufs=2))
        esp = top.enter_context(tc.tile_pool(name="esp",
                                             bufs=6 if deep else 4))
        ptp = top.enter_context(tc.tile_pool(name="ptp",
                                             bufs=10 if deep else 4))
        bcp = top.enter_context(tc.tile_pool(name="bcp", bufs=2))
        rcp = top.enter_context(tc.tile_pool(name="rcp", bufs=2))
        op = top.enter_context(tc.tile_pool(name="op", bufs=1))
        dscr = top.enter_context(tc.tile_pool(name="dscr", bufs=4,
                                              space="DRAM"))

        # ---- tiles for constants / persistent activations ----
        wq_sb = consts.tile([128, F, 128], f16, tag="wq")
        wk_sb = consts.tile([128, F, 128], f16, tag="wk")
        wv_sb = consts.tile([128, F, 128], f16, tag="wv")
        wo_sb = consts.tile([128, H], f16, tag="wo")
        qb_sb = consts.tile([128, 1], f32, tag="qb")
        kb_sb = consts.tile([128, 1], f32, tag="kb")
        m01f_sb = consts.tile([128, tt], f32, tag="m01f")
        m01v_sb = consts.tile([128, tt], f16, tag="m01v")
        ident = consts.tile([128, 128], f16, tag="ident")

        qT_sb = pers.tile([128, R], f16, tag="qT")
        kT_sb = pers.tile([128, B * sp], f16, tag="kT")
        v_nat = pers.tile([128, tt, 132], f16, tag="vn")
        ctxn = [pers.tile([128, S], f16, tag=f"ctxn{b}", name=f"ctxn{b}")
                for b in range(B)]
        ctx1 = [pers.tile([64, S], f16, tag=f"ctx1{b}", name=f"ctx1{b}")
                for b in range(B)]

        opr = opart.rearrange("(g p) hh -> p g hh", p=128)
        ebr = eb.rearrange("b qc (kb p) m -> p b qc kb m", p=128)
        xqr = xq.rearrange("(f p) r -> p f r", p=128)
        xkr = xk.rearrange("(f p) r -> p f r", p=128)
        xvr = xv.rearrange("(f p) r -> p f r", p=128)
        PIPE = 2
        op_pend = []

        # ---------- projection emitters ----------
        def proj_q(rc):
            xt = xpool.tile([128, F, 512], f16, tag="xt", name=f"xt_q{rc}")
            nc.sync.dma_start(xt, xqr[:, :, rc * 512:(rc + 1) * 512])
            ps = mm.tile([128, 512], f32, tag="sps", name=f"ps_q{rc}")
            for f in range(F):
                nc.tensor.matmul(ps, lhsT=wq_sb[:, f, :], rhs=xt[:, f, :],
                                 start=(f == 0), stop=(f == F - 1))
            nc.vector.tensor_scalar_add(
                qT_sb[:, rc * 512:(rc + 1) * 512], ps, qb_sb)

        def proj_k(b, ci):
            o, w = kchunks[ci]
            xt = xpool.tile([128, F, 512], f16, tag="xt", name=f"xt_k{b}_{ci}")
            nc.sync.dma_start(xt[:, :, 0:w], xkr[:, :, b * sp + o:b * sp + o + w])
            ps = mm.tile([128, 512], f32, tag="sps", name=f"ps_k{b}_{ci}")
            for f in range(F):
                nc.tensor.matmul(ps[:, 0:w], lhsT=wk_sb[:, f, :],
                                 rhs=xt[:, f, 0:w],
                                 start=(f == 0), stop=(f == F - 1))
            nc.vector.tensor_scalar_add(
                kT_sb[:, b * sp + o:b * sp + o + w], ps[:, 0:w], kb_sb)

        def proj_v(b, ci):
            o, w = kchunks[ci]
            xt = xpool.tile([128, F, 512], f16, tag="xt", name=f"xt_v{b}_{ci}")
            nc.sync.dma_start(xt[:, :, 0:w], xvr[:, :, b * sp + o:b * sp + o + w])
            ps = mm.tile([128, 512], f32, tag="sps", name=f"ps_v{b}_{ci}")
            for f in range(F):
                nc.tensor.matmul(ps[:, 0:w], lhsT=wv_sb[:, f, :],
                                 rhs=xt[:, f, 0:w],
                                 start=(f == 0), stop=(f == F - 1))
            vt = vtp.tile([128, 512], f16, tag="vt")
            nc.vector.tensor_copy(vt[:, 0:w], ps[:, 0:w])
            for i in range(w // 128):
                t = b * kbe + o // 128 + i
                tp = mm.tile([128, 128], f16, tag="sps", name=f"tp{t}")
                nc.tensor.transpose(tp, vt[:, i * 128:(i + 1) * 128], ident)
                for h in range(HPC):
                    nc.vector.tensor_scalar_mul(
                        v_nat[:, t, h * 66:h * 66 + 64],
                        tp[:, h * 64:(h + 1) * 64],
                        m01f_sb[:, t:t + 1])

        def make_ebq(qc, b):
            return ebp.tile([128, kbe, HPC * 512], f16, tag="eb",
                            name=f"ebq{qc}_{b}")

        def load_ebq(ebq, qc, b, gi):
            g, n = grans[gi]
            nc.sync.dma_start(ebq[:, g:g + n, :], ebr[:, b, qc, g:g + n, :])

        def mergef(d):
            """Clamp filler positions to valid kb range and merge."""
            out = {}
            for pos, fns in d.items():
                p = min(pos, kbe - 1)
                out.setdefault(p, []).extend(fns)
            return out

        # ---------- attention chunk emitter ----------
        def attn(qc, b, ebq, fillers=None, pipe=PIPE):
            cvp = [cvp_pool.tile([65, 512], f32, tag="cv",
                                 name=f"cv{qc}_{b}_{h}")
                   for h in range(HPC)]

            def emit_pv(ptt, kb):
                for h in range(HPC):
                    nc.tensor.matmul(
                        cvp[h],
                        lhsT=v_nat[:, b * kbe + kb, h * 66:h * 66 + 65],
                        rhs=ptt[:, h, :],
                        start=(kb == 0), stop=(kb == kbe - 1))

            pend = []
            for kb in range(kbe):
                sps = mm.tile([128, HPC, 512], f32, tag="sps",
                              name=f"sps{qc}_{kb}_{b}")
                for h in range(HPC):
                    nc.tensor.matmul(
                        sps[:, h, :],
                        lhsT=kT_sb[h * 64:(h + 1) * 64,
                                   b * sp + kb * 128:b * sp + (kb + 1) * 128],
                        rhs=qT_sb[h * 64:(h + 1) * 64,
                                  b * S + qc * 512:b * S + (qc + 1) * 512],
                        start=True, stop=True)
                est = esp.tile([128, HPC, 512], f16, tag="es")
                nc.scalar.activation(est, sps, func=Exp, scale=SCALE)
                ptt = ptp.tile([128, HPC, 512], f16, tag="pt")
                ebt = ebq[:, kb, :].rearrange("p (i q) -> p i q", i=HPC)
                eng = nc.gpsimd if kb % 3 == 2 else nc.vector
                eng.tensor_mul(ptt, est, ebt)
                pend.append((ptt, kb))
                if fillers and kb in fillers:
                    for fn in fillers[kb]:
                        fn()
                if len(pend) > pipe:
                    emit_pv(*pend.pop(0))
            for args in pend:
                emit_pv(*args)

            # previous chunk's out-projection (inputs long since ready)
            while len(op_pend) > 1:
                op_pend.pop(0)()

            # evacuate ctx from PSUM (frees cv banks before the broadcast
            # DMA round-trip)
            cvs = bcp.tile([64, HPC, 512], f32, tag="cvs",
                           name=f"cvs{qc}_{b}")
            rc_sb = rcp.tile([65, HPC, 512], f32, tag="rc")
            for h in range(HPC):
                nc.vector.reciprocal(rc_sb[64:65, h, :], cvp[h][64:65, :])
                nc.vector.tensor_copy(cvs[:, h, :], cvp[h][0:64, :])

            # normalize: ctxn = ctx.T * (1/den)
            scr = dscr.tile([1, HPC, 512], f32, tag="scr",
                            name=f"scr{qc}_{b}")
            nc.sync.dma_start(scr, rc_sb[64:65, :, :])
            bc = bcp.tile([64, HPC, 512], f32, tag="bc")
            nc.sync.dma_start(bc, scr.to_broadcast((64, HPC, 512)))
            nc.gpsimd.tensor_mul(
                ctxn[b][0:64, qc * 512:(qc + 1) * 512], cvs[:, 0, :],
                bc[:, 0, :])
            nc.gpsimd.tensor_mul(
                ctx1[b][:, qc * 512:(qc + 1) * 512], cvs[:, 1, :],
                bc[:, 1, :])
            nc.sync.dma_start(
                ctxn[b][64:128, qc * 512:(qc + 1) * 512],
                ctx1[b][:, qc * 512:(qc + 1) * 512])

            def emit_op(qc=qc, b=b):
                ob_g = op.tile([128, QC, H], f16, tag="ob",
                               name=f"ob{qc}_{b}")
                for ri in range(QC):
                    rb = qc * QC + ri
                    po = mm.tile([128, HPC, 512], f32, tag="sps",
                                 name=f"po{qc}_{b}_{ri}")
                    lhsT = ctxn[b][:, rb * 128:(rb + 1) * 128]
                    nc.tensor.matmul(po[:, 0, :], lhsT=lhsT,
                                     rhs=wo_sb[:, 0:512],
                                     start=True, stop=True)
                    nc.tensor.matmul(po[:, 1, :], lhsT=lhsT,
                                     rhs=wo_sb[:, 512:1024],
                                     start=True, stop=True)
                    # split PSUM evacuation between DVE and the scalar
                    # engine (Act has slack once exp shrank with KBE)
                    dst = ob_g[:, ri, :].rearrange("p (i j) -> p i j", i=2)
                    if ri % 2 == 0:
                        nc.vector.tensor_copy(dst, po)
                    else:
                        nc.scalar.copy(dst, po)
                g0 = b * (S // 128) + qc * QC
                nc.sync.dma_start(opr[:, g0:g0 + QC, :], ob_g)
            op_pend.append(emit_op)

        # ---------- interleaved schedule ----------
        # DMA is effectively a serial queue in emission order and each
        # engine executes in emission order, so both are engineered
        # jointly via mid-chunk "fillers".
        nc.sync.dma_start(wq_sb, wq.rearrange("(f p) j -> p f j", p=128))
        nc.sync.dma_start(qb_sb, qb)
        proj_q(0)
        nc.sync.dma_start(wk_sb, wk.rearrange("(f p) j -> p f j", p=128))
        nc.sync.dma_start(kb_sb, kb_)
        nc.sync.dma_start(m01f_sb, m01f)
        nc.sync.dma_start(m01v_sb, m01h)
        make_identity(nc, ident)
        m01v_col = m01v_sb.rearrange("p (t o) -> p t o", o=1)
        nc.vector.tensor_copy(v_nat[:, :, 64:65], m01v_col)
        nc.vector.tensor_copy(v_nat[:, :, 130:131], m01v_col)
        for ci in range(len(kchunks)):
            proj_k(0, ci)
        ebq00 = make_ebq(0, 0)
        for gi in range(len(grans)):
            load_ebq(ebq00, 0, 0, gi)
        nc.sync.dma_start(wv_sb, wv.rearrange("(f p) j -> p f j", p=128))
        ebq01 = make_ebq(0, 1)
        nk = len(kchunks)
        ng = len(grans)

        if not deep:
            # Fallback for little/no masking: plain sequential schedule,
            # shallow pipe, correctness over peak throughput.
            for ci in range(nk):
                proj_v(0, ci)
            proj_q(4)
            for ci in range(nk):
                proj_k(1, ci)
            for ci in range(nk):
                proj_v(1, ci)
            for gi in range(ng):
                load_ebq(ebq01, 0, 1, gi)
            nc.sync.dma_start(wo_sb, wo)
            for rc in (1, 5, 2, 6, 3, 7):
                proj_q(rc)
            attn(0, 0, ebq00)
            attn(0, 1, ebq01)
            for qc in range(1, QC):
                for b in range(B):
                    ebq = make_ebq(qc, b)
                    for gi in range(ng):
                        load_ebq(ebq, qc, b, gi)
                    attn(qc, b, ebq)
            for fn in op_pend:
                fn()
            return

        # b0's v-projections ride as late fillers (their loads queue after
        # the k/q data that feeds the next chunk's scores); the deep pipe
        # defers c00's PV into the drain, past the v writes.
        attn(0, 0, ebq00, mergef({
            1: [lambda: proj_q(4)],
            3: [lambda: proj_k(1, 0)],
            5: [(lambda ci=ci: proj_k(1, ci)) for ci in range(1, nk)],
            7: [lambda: proj_v(0, 0)],
            8: [(lambda ci=ci: proj_v(0, ci)) for ci in range(1, nk)]
               + [(lambda gi=gi: load_ebq(ebq01, 0, 1, gi))
                  for gi in range(ng)],
        }), pipe=8)
        ebq10 = make_ebq(1, 0)
        attn(0, 1, ebq01, mergef({
            1: [lambda: nc.sync.dma_start(wo_sb, wo),
                lambda: proj_q(1)],
            5: [lambda: proj_v(1, 0)],
            7: [(lambda ci=ci: proj_v(1, ci)) for ci in range(1, nk)],
            8: [lambda: load_ebq(ebq10, 1, 0, 0)],
        }), pipe=8)
        ebq11 = make_ebq(1, 1)
        attn(1, 0, ebq10, mergef({
            1: [(lambda gi=gi: load_ebq(ebq10, 1, 0, gi))
                for gi in range(1, ng)],
            3: [lambda: proj_q(5)],
            5: [(lambda gi=gi: load_ebq(ebq11, 1, 1, gi))
                for gi in range(ng)],
        }))
        ebq20 = make_ebq(2, 0)
        attn(1, 1, ebq11, mergef({
            1: [lambda: proj_q(2)],
            3: [(lambda gi=gi: load_ebq(ebq20, 2, 0, gi))
                for gi in range(ng)],
        }))
        ebq21 = make_ebq(2, 1)
        attn(2, 0, ebq20, mergef({
            1: [lambda: proj_q(6)],
            3: [(lambda gi=gi: load_ebq(ebq21, 2, 1, gi))
                for gi in range(ng)],
        }))
        ebq30 = make_ebq(3, 0)
        attn(2, 1, ebq21, mergef({
            1: [lambda: proj_q(3)],
            3: [(lambda gi=gi: load_ebq(ebq30, 3, 0, gi))
                for gi in range(ng)],
        }))
        ebq31 = make_ebq(3, 1)
        attn(3, 0, ebq30, mergef({
            1: [lambda: proj_q(7)],
            3: [(lambda gi=gi: load_ebq(ebq31, 3, 1, gi))
                for gi in range(ng)],
        }))
        attn(3, 1, ebq31)
        for fn in op_pend:
            fn()


def get_module(reps=1, kbe=None):
    if kbe is None:
        kbe = _LAST_KBE if _LAST_KBE is not None else DEFAULT_KBE
    key = f"nc{reps}_{kbe}"
    if key not in _CACHE:
        _CACHE[key] = _build_module(reps, kbe)
    return _CACHE[key]


def make_in_maps(query, key, value, key_padding_mask, bias,
                 q_w, q_b, k_w, k_b, v_w, v_b, o_w, o_b):
    global _LAST_KBE
    f16 = np.float16
    kpm = np.asarray(key_padding_mask)
    kbe = _kbe_for(kpm)
    _LAST_KBE = kbe
    sp = kbe * 128
    tt = B * kbe

    xq_t = np.ascontiguousarray(
        np.asarray(query).reshape(R, H).T).astype(f16)

    idx = [np.flatnonzero(~kpm[b]) for b in range(B)]
    cnt = [len(ix) for ix in idx]
    kg = np.zeros((B, sp, H), np.float32)
    vg = np.zeros((B, sp, H), np.float32)
    for b in range(B):
        kg[b, :cnt[b]] = np.asarray(key)[b, idx[b]]
        vg[b, :cnt[b]] = np.asarray(value)[b, idx[b]]
    xk_t = np.ascontiguousarray(kg.reshape(B * sp, H).T).astype(f16)
    xv_t = np.ascontiguousarray(vg.reshape(B * sp, H).T).astype(f16)

    # m01 columns t = b*kbe + kb: 1.0 for real (compacted) keys, 0 for pad
    m01 = np.zeros((128, tt), np.float32)
    for b in range(B):
        for kb in range(kbe):
            base = kb * 128
            n = min(max(cnt[b] - base, 0), 128)
            m01[:n, b * kbe + kb] = 1.0
    m01_f32 = np.ascontiguousarray(m01)
    m01v = m01.astype(f16)

    ebias = np.exp(np.asarray(bias[0], np.float32))  # [NH, S, S]

    in_maps = []
    for c in range(NCORES):
        hs = slice(c * CW, (c + 1) * CW)
        # eb layout [b, qc, j, i*512+qi] = exp(bias)[h, qc*512+qi, idx_b[j]]
        ebt = np.zeros((B, QC, sp, HPC, 512), f16)
        for i in range(HPC):
            h = c * HPC + i
            for b in range(B):
                g = ebias[h][:, idx[b]].T.astype(f16)      # [cnt, S]
                ebt[b, :, :cnt[b], i, :] = (
                    g.reshape(cnt[b], QC, 512).transpose(1, 0, 2))
        ebt = ebt.reshape(B, QC, sp, HPC * 512)
        in_maps.append({
            "xq_t": xq_t, "xk_t": xk_t, "xv_t": xv_t,
            "wq_t": np.ascontiguousarray(np.asarray(q_w)[hs].T).astype(f16),
            "wk_t": np.ascontiguousarray(np.asarray(k_w)[hs].T).astype(f16),
            "wv_t": np.ascontiguousarray(np.asarray(v_w)[hs].T).astype(f16),
            "wo_t": np.ascontiguousarray(np.asarray(o_w)[:, hs].T).astype(f16),
            "qb_col": np.asarray(q_b, np.float32)[hs].reshape(CW, 1).copy(),
            "kb_col": np.asarray(k_b, np.float32)[hs].reshape(CW, 1).copy(),
            "eb_t": ebt,
            "m01_f32": m01_f32,
            "m01_v": m01v,
        })
    return in_maps


def assemble_output(results, v_b, o_w, o_b):
    acc = np.zeros((R, H), np.float32)
    for res in results:
        acc += np.asarray(res["o_part"], np.float32)
    corr = np.asarray(v_b, np.float32) @ np.asarray(o_w, np.float32).T \
        + np.asarray(o_b, np.float32)
    acc += corr[None, :]
    return acc.reshape(B, S, H).astype(np.float32)


def kernel(**inputs):
    from concourse.bass_utils import run_bass_kernel_spmd

    in_maps = make_in_maps(**inputs)
    nc = get_module(kbe=_LAST_KBE)
    res = run_bass_kernel_spmd(nc, in_maps, list(range(NCORES)))
    return assemble_output(res.results, inputs["v_b"], inputs["o_w"],
                           inputs["o_b"])
